# revision 11
# baseline (speedup 1.0000x reference)
"""3-layer GCN (PyG GCNConv semantics) on 8 Trainium2 NeuronCores.

Contract: kernel(**inputs) takes the FULL inputs (x [50000,128] f32,
edge_index [2,800000] int, W1/b1/W2/b2/W3/b3) and returns the FULL
output [50000, 64] f32.

Sharding: nodes are partitioned across the 8 cores by destination
(6250 rows each).  Per layer, each core scales its local rows by
dinv = deg^-1/2 and casts to fp16; an AllGather replicates the full
scaled node-feature table to every core's DRAM.  Each core then
computes its destination shard:

    agg[d,:]  = sum_{edges (s,d)} t[s,:]          (t = dinv * h, one-hot S)
    mm[d,:]   = (dinv[d] * agg[d,:]) @ W + b
    t_next[d] = relu(dinv[d] * mm[d,:])           (inner layers)
    y[d]      = mm[d,:]                           (last layer)

The per-edge norm dinv[s]*dinv[d] is recovered exactly by the source-side
pre-scale and destination-side post-scale, so the per-edge selector
matrices S[e,d] = (d == dst_local_e) are PURE one-hot {0,1}.  They depend
only on the (fixed) graph, so each S tile is built exactly once per
pipeline (a single DVE tensor_scalar is_equal, fp8 output) into a
persistent SBUF cache and reused by all three layers; the PE aggregation
reads them as the streaming rhs operand (fp16 x fp8 matmul).

The sparse aggregation per 128-dst block: dma_gather of the 128 source
rows of each edge tile (fp16, 256B rows) and a PE matmul
agg[f,d] += M[e,f].T @ S[e,d] accumulated in PSUM over the ~19 edge
tiles of the block.  dma_gather indices are int16, so the table is
addressed as two halves (<25600 / >=25600 local-row split), which also
lets each half's AllGather fire as soon as the first/second half of the
blocks finished (the per-edge metadata bakes the node->table-row map).
"""

import numpy as np

FEAT = 128
N_CORES = 8
SPLIT_BLK = 25        # blocks 0..24 -> table region A, 25.. -> region B
CHUNK_BLOCKS = 2


# ---------------------------------------------------------------- host side

def preprocess(edge_index: np.ndarray, n_nodes: int, n_cores: int = N_CORES,
               chunk_blocks: int = CHUNK_BLOCKS):
    """Uniform SPMD tile schedule + per-core gather/metadata arrays."""
    src = np.asarray(edge_index[0], dtype=np.int64)
    dst = np.asarray(edge_index[1], dtype=np.int64)
    loops = np.arange(n_nodes, dtype=np.int64)
    s = np.concatenate([src, loops])
    d = np.concatenate([dst, loops])
    deg = np.bincount(d, minlength=n_nodes).astype(np.float64)  # >= 1
    dinv = (1.0 / np.sqrt(deg)).astype(np.float32)

    S_pc = n_nodes // n_cores
    assert S_pc * n_cores == n_nodes
    nblocks = (S_pc + 127) // 128
    split_blk = min(SPLIT_BLK, (nblocks + 1) // 2)
    ROWS_A = min(split_blk * 128, S_pc)      # local rows in table region A
    ROWS_B = S_pc - ROWS_A
    assert n_cores * ROWS_A < 32768 and n_cores * ROWS_B < 32768

    # node -> table row (region, row): node = q*S_pc + p
    q = s // S_pc
    p = s - q * S_pc
    in_a = p < ROWS_A
    trow = np.where(in_a, q * ROWS_A + p, q * ROWS_B + (p - ROWS_A))

    core_of = d // S_pc
    dloc = d - core_of * S_pc
    blk = dloc // 128
    dst_local = (dloc - blk * 128).astype(np.int32)
    is_hi = (~in_a).astype(np.int64)

    counts = np.zeros((n_cores, nblocks, 2), dtype=np.int64)
    np.add.at(counts, (core_of, blk, is_hi), 1)
    T_lo = -(-counts[:, :, 0].max(axis=0) // 128)
    T_hi = -(-counts[:, :, 1].max(axis=0) // 128)

    order = np.lexsort((is_hi, blk, core_of))
    trow_o, dl_o = trow[order], dst_local[order]
    co_o, blk_o, hi_o = core_of[order], blk[order], is_hi[order]

    n_tiles_lo = int(T_lo.sum())
    n_tiles_hi = int(T_hi.sum())
    cum_lo = np.concatenate([[0], np.cumsum(T_lo)])
    cum_hi = np.concatenate([[0], np.cumsum(T_hi)])

    key = (co_o * nblocks + blk_o) * 2 + hi_o
    bounds = np.searchsorted(key, np.arange(n_cores * nblocks * 2 + 1))
    per_core = []
    for r in range(n_cores):
        idx_lo = np.zeros(128 * n_tiles_lo, dtype=np.int16)
        idx_hi = np.zeros(128 * n_tiles_hi, dtype=np.int16)
        dloc_lo = np.full((128, max(n_tiles_lo, 1)), -1.0, dtype=np.float32)
        dloc_hi = np.full((128, max(n_tiles_hi, 1)), -1.0, dtype=np.float32)
        for b in range(nblocks):
            for h in range(2):
                k = (r * nblocks + b) * 2 + h
                lo_, hi_ = bounds[k], bounds[k + 1]
                cnt = hi_ - lo_
                t0 = cum_lo[b] if h == 0 else cum_hi[b]
                iarr = idx_lo if h == 0 else idx_hi
                darr = dloc_lo if h == 0 else dloc_hi
                iarr[128 * t0: 128 * t0 + cnt] = trow_o[lo_:hi_].astype(np.int16)
                e = np.arange(cnt)
                darr[e % 128, t0 + e // 128] = dl_o[lo_:hi_]

        def wrap(a):  # [n] -> [128, n//16]; idx i at [i%16 + 16k, i//16]
            n = len(a)
            if n == 0:
                return np.zeros((128, 1), dtype=np.int16)
            w = a.reshape(n // 16, 16).T
            return np.tile(w, (8, 1)).copy()

        # per-block dinv of the core's local (destination) nodes
        dv = np.ones(nblocks * 128, dtype=np.float32)
        dv[:S_pc] = dinv[r * S_pc:(r + 1) * S_pc]
        per_core.append(dict(
            idx_lo=wrap(idx_lo), idx_hi=wrap(idx_hi),
            dloc_lo=dloc_lo, dloc_hi=dloc_hi,
            dinv_blk=dv.reshape(nblocks, 128).T.copy(),  # [128, nblocks]
        ))

    chunks = [list(range(c, min(c + chunk_blocks, nblocks)))
              for c in range(0, nblocks, chunk_blocks)]
    sched = dict(
        n_nodes=n_nodes, n_cores=n_cores, S_pc=S_pc, nblocks=nblocks,
        split_blk=split_blk, ROWS_A=ROWS_A, ROWS_B=ROWS_B,
        T_lo=T_lo.astype(int), T_hi=T_hi.astype(int),
        cum_lo=cum_lo.astype(int), cum_hi=cum_hi.astype(int),
        n_tiles_lo=n_tiles_lo, n_tiles_hi=n_tiles_hi, chunks=chunks,
    )
    return sched, per_core


def make_inputs(sched, per_core, x, Ws, bs):
    n_cores, S_pc = sched["n_cores"], sched["S_pc"]
    iota = np.tile(np.arange(128, dtype=np.float32)[None, :], (128, 1))
    in_maps = []
    for r in range(n_cores):
        m = dict(
            x_shard=np.ascontiguousarray(x[r * S_pc:(r + 1) * S_pc]).astype(np.float32),
            iota=iota,
            idx_lo=per_core[r]["idx_lo"], idx_hi=per_core[r]["idx_hi"],
            dloc_lo=per_core[r]["dloc_lo"], dloc_hi=per_core[r]["dloc_hi"],
            dinv_blk=per_core[r]["dinv_blk"],
        )
        for i, (W, b) in enumerate(zip(Ws, bs)):
            m[f"W{i}"] = np.asarray(W).astype(np.float16)
            m[f"b{i}"] = np.tile(np.asarray(b, dtype=np.float32)[None, :], (128, 1))
        in_maps.append(m)
    return in_maps


# ---------------------------------------------------------------- device side

def build_nc(sched, fos=(128, 128, 64), n_cores=None, model=False,
             compile=True, single_packet=False, max_gather_idx=None,
             scratch=None, reps=1, probe=None,
             swdge_queues=4, agg_bufs=4, mbuf_bufs=4):
    """model=True: single-core cost-model variant (AllGather replaced by a
    local DMA) for TimelineSim.  reps>1 replicates the whole pipeline for
    on-hardware delta timing; the per-pipeline S-tile builds are re-issued
    every rep so the marginal rep is a faithful single-pipeline time."""
    import concourse.bacc as bacc
    import concourse.tile as tile
    import concourse.mybir as mybir

    f16, f32, i16 = mybir.dt.float16, mybir.dt.float32, mybir.dt.int16
    f8 = mybir.dt.float8e4
    N, S_pc = sched["n_nodes"], sched["S_pc"]
    nblocks = sched["nblocks"]
    ROWS_A = sched["ROWS_A"]
    split_blk = sched["split_blk"]
    T_lo, T_hi = sched["T_lo"], sched["T_hi"]
    cum_lo, cum_hi = sched["cum_lo"], sched["cum_hi"]
    NT_lo, NT_hi = sched["n_tiles_lo"], sched["n_tiles_hi"]
    chunks = sched["chunks"]
    n_cores = n_cores or sched["n_cores"]
    n_layers = len(fos)
    NA = n_cores * ROWS_A            # rows in table region A
    NB = N - NA
    assert NB > 0, "need a non-empty table region B"

    kw = {}
    if scratch:
        kw["dynamic_dma_scratch_size"] = scratch
    if swdge_queues > 1:
        kw["num_swdge_queues"] = swdge_queues
    nc = bacc.Bacc("TRN2", target_bir_lowering=False, debug=False,
                   num_devices=n_cores, **kw)

    x_shard = nc.dram_tensor("x_shard", [S_pc, FEAT], f32, kind="ExternalInput")
    iota_in = nc.dram_tensor("iota", [128, 128], f32, kind="ExternalInput")
    idx_lo_in = nc.dram_tensor("idx_lo", [128, max(NT_lo * 8, 1)], i16, kind="ExternalInput")
    idx_hi_in = nc.dram_tensor("idx_hi", [128, max(NT_hi * 8, 1)], i16, kind="ExternalInput")
    dloc_lo_in = nc.dram_tensor("dloc_lo", [128, max(NT_lo, 1)], f32, kind="ExternalInput")
    dloc_hi_in = nc.dram_tensor("dloc_hi", [128, max(NT_hi, 1)], f32, kind="ExternalInput")
    dinv_in = nc.dram_tensor("dinv_blk", [128, nblocks], f32, kind="ExternalInput")
    W_in = [nc.dram_tensor(f"W{i}", [FEAT, fos[i]], f16, kind="ExternalInput")
            for i in range(n_layers)]
    b_in = [nc.dram_tensor(f"b{i}", [128, fos[i]], f32, kind="ExternalInput")
            for i in range(n_layers)]
    y_out = nc.dram_tensor("y", [S_pc, fos[-1]], f32, kind="ExternalOutput")

    rg = [list(range(n_cores))]

    with tile.TileContext(nc) as tc:
        with (
            tc.tile_pool(name="const", bufs=1) as cpool,
            tc.tile_pool(name="sb", bufs=3) as sb,
            tc.tile_pool(name="mbuf", bufs=mbuf_bufs) as mbuf,
            tc.tile_pool(name="psum_agg", bufs=agg_bufs, space="PSUM") as psum_agg,
            tc.tile_pool(name="psum_mm", bufs=2, space="PSUM") as psum_mm,
            tc.tile_pool(name="dram", bufs=2, space="DRAM") as dram,
        ):
            # --- constants, loaded once
            iota = cpool.tile([128, 128], f32)
            nc.sync.dma_start(out=iota[:], in_=iota_in[:])
            idx_lo = cpool.tile([128, max(NT_lo * 8, 1)], i16)
            nc.sync.dma_start(out=idx_lo[:], in_=idx_lo_in[:])
            idx_hi = cpool.tile([128, max(NT_hi * 8, 1)], i16)
            nc.sync.dma_start(out=idx_hi[:], in_=idx_hi_in[:])
            dloc_lo = cpool.tile([128, max(NT_lo, 1)], f32)
            nc.sync.dma_start(out=dloc_lo[:], in_=dloc_lo_in[:])
            dloc_hi = cpool.tile([128, max(NT_hi, 1)], f32)
            nc.sync.dma_start(out=dloc_hi[:], in_=dloc_hi_in[:])
            dinv = cpool.tile([128, nblocks], f32)
            nc.sync.dma_start(out=dinv[:], in_=dinv_in[:])
            Wt, bt = [], []
            for i in range(n_layers):
                w = cpool.tile([FEAT, fos[i]], f16, tag=f"W{i}")
                nc.sync.dma_start(out=w[:], in_=W_in[i][:])
                Wt.append(w)
                b = cpool.tile([128, fos[i]], f32, tag=f"b{i}")
                nc.sync.dma_start(out=b[:], in_=b_in[i][:])
                bt.append(b)

            # persistent SBUF cache of all one-hot S tiles (fp8): lo tiles at
            # col t*128, hi tiles at (NT_lo+t)*128
            s_cache = cpool.tile([128, (NT_lo + NT_hi) * 128], f8, tag="scache")

            def S_of(half, col):
                o = (col if half == 0 else NT_lo + col) * 128
                return s_cache[:, o:o + 128]

            gq = [0]

            def emit_gather(M, src_ap, idx_sb, t0, nt):
                if probe == "nogather":
                    nc.sync.dma_start(out=M[:],
                                      in_=src_ap[0:128 * nt, :].rearrange(
                                          "(p t) f -> p t f", p=128))
                    return
                step = nt if not max_gather_idx else max(1, max_gather_idx // 128)
                for s0 in range(0, nt, step):
                    sn = min(step, nt - s0)
                    o16 = (t0 + s0) * 8
                    gq[0] = (gq[0] + 1) % swdge_queues
                    nc.gpsimd.dma_gather(
                        out_ap=M[:, s0:s0 + sn, :], in_ap=src_ap,
                        idxs_ap=idx_sb[:, o16:o16 + sn * 8],
                        num_idxs=128 * sn, num_idxs_reg=128 * sn,
                        elem_size=FEAT, single_packet=single_packet,
                        queue_num=gq[0])

            for rep in range(reps):
                # --- build all one-hot S tiles once per pipeline (DVE)
                if probe != "noS":
                    for colh in range(NT_lo + NT_hi):
                        half, col = (0, colh) if colh < NT_lo else (1, colh - NT_lo)
                        dl = dloc_lo if half == 0 else dloc_hi
                        nc.vector.tensor_scalar(
                            S_of(half, col), iota[:], dl[:, col:col + 1], None,
                            mybir.AluOpType.is_equal)
                elif rep == 0:
                    nc.vector.tensor_scalar(
                        s_cache[:, 0:128], iota[:], dloc_lo[:, 0:1], None,
                        mybir.AluOpType.is_equal)

                # --- layer 0 input: scale by dinv and cast to fp16
                ag_in_a = dram.tile([ROWS_A, FEAT], f16, tag="ag_in_a")
                ag_in_b = dram.tile([S_pc - ROWS_A, FEAT], f16, tag="ag_in_b")
                for b in range(nblocks):
                    r0, r1 = b * 128, min((b + 1) * 128, S_pc)
                    rows = r1 - r0
                    xt = sb.tile([128, FEAT], f32, tag="xcast_in")
                    nc.sync.dma_start(out=xt[:rows, :], in_=x_shard[r0:r1, :])
                    xh = sb.tile([128, FEAT], f16, tag="xcast_out")
                    nc.vector.tensor_scalar(
                        xh[:rows, :], xt[:rows, :], dinv[:rows, b:b + 1], None,
                        mybir.AluOpType.mult)
                    if b < split_blk:
                        nc.sync.dma_start(out=ag_in_a[r0:r1, :], in_=xh[:rows, :])
                    else:
                        nc.sync.dma_start(
                            out=ag_in_b[r0 - ROWS_A:r1 - ROWS_A, :],
                            in_=xh[:rows, :])

                for l in range(n_layers):
                    fo = fos[l]
                    # --- allgather: region A (blocks < split_blk), region B
                    tbl_a = dram.tile([NA, FEAT], f16, tag="tbl_a")
                    tbl_b = dram.tile([NB, FEAT], f16, tag="tbl_b")
                    if model:
                        nc.sync.dma_start(out=tbl_a[0:ROWS_A, :], in_=ag_in_a[:])
                        nc.sync.dma_start(out=tbl_b[0:S_pc - ROWS_A, :], in_=ag_in_b[:])
                    else:
                        nc.gpsimd.collective_compute(
                            "AllGather", mybir.AluOpType.bypass,
                            replica_groups=rg,
                            ins=[ag_in_a[:].opt()], outs=[tbl_a[:].opt()])
                        nc.gpsimd.collective_compute(
                            "AllGather", mybir.AluOpType.bypass,
                            replica_groups=rg,
                            ins=[ag_in_b[:].opt()], outs=[tbl_b[:].opt()])
                    if l + 1 < n_layers:
                        ag_in_a = dram.tile([ROWS_A, FEAT], f16, tag="ag_in_a")
                        ag_in_b = dram.tile([S_pc - ROWS_A, FEAT], f16, tag="ag_in_b")

                    for chunk in chunks:
                        ctlo = int(sum(T_lo[b] for b in chunk))
                        cthi = int(sum(T_hi[b] for b in chunk))
                        M_lo = M_hi = None
                        if ctlo:
                            M_lo = mbuf.tile([128, ctlo, FEAT], f16, tag="Mlo")
                            emit_gather(M_lo, tbl_a[:], idx_lo,
                                        int(cum_lo[chunk[0]]), ctlo)
                        if cthi:
                            M_hi = mbuf.tile([128, cthi, FEAT], f16, tag="Mhi")
                            emit_gather(M_hi, tbl_b[:], idx_hi,
                                        int(cum_hi[chunk[0]]), cthi)
                        if probe == "only_gather":
                            continue
                        for b in chunk:
                            tiles = (
                                [(0, M_lo, cum_lo[b] - cum_lo[chunk[0]] + k,
                                  cum_lo[b] + k)
                                 for k in range(T_lo[b])] +
                                [(1, M_hi, cum_hi[b] - cum_hi[chunk[0]] + k,
                                  cum_hi[b] + k)
                                 for k in range(T_hi[b])])
                            agg = psum_agg.tile([128, 128], f32)
                            for j, (half, Mt, sl, col) in enumerate(tiles):
                                S = (S_of(half, int(col)) if probe != "noS"
                                     else s_cache[:, 0:128])
                                nc.tensor.matmul(agg[:], Mt[:, sl, :], S,
                                                 start=(j == 0),
                                                 stop=(j == len(tiles) - 1))
                            aggT = sb.tile([128, 128], f16, tag="aggT")
                            nc.scalar.copy(aggT[:], agg[:])
                            mm = psum_mm.tile([128, fo], f32, tag="mm")
                            nc.tensor.matmul(mm[:], aggT[:], Wt[l][:],
                                             start=True, stop=True)
                            r0, r1 = b * 128, min((b + 1) * 128, S_pc)
                            rows = r1 - r0
                            # hs = dinv[d]*mm + b
                            hs = sb.tile([128, fo], f32, tag="hsum")
                            nc.vector.tensor_scalar(
                                hs[:], mm[:], dinv[:, b:b + 1], None,
                                mybir.AluOpType.mult)
                            hb = sb.tile([128, fo], f32 if l + 1 == n_layers else f16,
                                         tag="hbias")
                            nc.vector.tensor_tensor(
                                hb[:], hs[:], bt[l][:], mybir.AluOpType.add)
                            if l + 1 < n_layers:
                                # t_next = relu(dinv[d] * hb)
                                h = sb.tile([128, fo], f16, tag="hout")
                                nc.scalar.activation(
                                    h[:], hb[:],
                                    mybir.ActivationFunctionType.Relu,
                                    scale=dinv[:, b:b + 1])
                                if b < split_blk:
                                    nc.sync.dma_start(out=ag_in_a[r0:r1, :],
                                                      in_=h[:rows, :])
                                else:
                                    nc.sync.dma_start(
                                        out=ag_in_b[r0 - ROWS_A:r1 - ROWS_A, :],
                                        in_=h[:rows, :])
                            else:
                                nc.sync.dma_start(out=y_out[r0:r1, :],
                                                  in_=hb[:rows, :])
    if compile:
        nc.compile()
    return nc


# ---------------------------------------------------------------- entry point

_CACHE = {}


def kernel(x, edge_index, W1, b1, W2, b2, W3, b3):
    import sys
    if "/opt/trn_rl_repo" not in sys.path:
        sys.path.insert(0, "/opt/trn_rl_repo")
    from concourse import bass_utils

    x = np.asarray(x)
    edge_index = np.asarray(edge_index)
    Ws = [np.asarray(W1), np.asarray(W2), np.asarray(W3)]
    bs = [np.asarray(b1), np.asarray(b2), np.asarray(b3)]
    n = x.shape[0]

    key = (n, edge_index.shape[1])
    if key in _CACHE and np.array_equal(_CACHE[key][0], edge_index):
        _, sched, per_core, nc = _CACHE[key]
    else:
        sched, per_core = preprocess(edge_index, n, N_CORES)
        nc = build_nc(sched, fos=(W1.shape[1], W2.shape[1], W3.shape[1]))
        _CACHE[key] = (edge_index.copy(), sched, per_core, nc)

    in_maps = make_inputs(sched, per_core, x, Ws, bs)
    res = bass_utils.run_bass_kernel_spmd(nc, in_maps,
                                          core_ids=list(range(N_CORES)))
    out = np.concatenate([res.results[r]["y"] for r in range(N_CORES)], axis=0)
    return out.astype(np.float32)


# revision 12
# speedup vs baseline: 1.1808x; 1.1808x over previous
"""3-layer GCN (PyG GCNConv semantics) on 8 Trainium2 NeuronCores.

Contract: kernel(**inputs) takes the FULL inputs (x [50000,128] f32,
edge_index [2,800000] int, W1/b1/W2/b2/W3/b3) and returns the FULL
output [50000, 64] f32.

Sharding: nodes are partitioned across the 8 cores by destination
(6250 rows each).  Per layer, each core scales its local rows by
dinv = deg^-1/2 and casts to fp16; an AllGather replicates the full
scaled node-feature table to every core's DRAM.  Each core then
computes its destination shard:

    agg[d,:]  = sum_{edges (s,d)} t[s,:]          (t = dinv * h, one-hot S)
    mm[d,:]   = (dinv[d] * agg[d,:]) @ W + b
    t_next[d] = relu(dinv[d] * mm[d,:])           (inner layers)
    y[d]      = mm[d,:]                           (last layer)

The per-edge norm dinv[s]*dinv[d] is recovered exactly by the source-side
pre-scale and destination-side post-scale, so the per-edge selector
matrices S[e,d] = (d == dst_local_e) are PURE one-hot {0,1}.  They depend
only on the (fixed) graph, so S tiles are built once per pipeline into a
persistent fp8 SBUF cache (~19 tiles per DVE op via stride-0 broadcast
is_equal) interleaved with the first layer's chunks, and reused by the
later layers; the PE aggregation reads them as the streaming rhs operand
(fp16 x fp8 matmul).

The sparse aggregation per 128-dst block: dma_gather of the 128 source
rows of each edge tile (fp16, 256B rows) and a PE matmul
agg[f,d] += M[e,f].T @ S[e,d] accumulated in PSUM over the ~18 edge
tiles of the block.  dma_gather indices are int16, so the table is
addressed as two regions (A: blocks < 31, B: rest); region A's AllGather
fires as soon as the A blocks finish, and the A-side ("lo") gathers of
the next layer run LO_LAG chunks ahead on their own SWDGE queues while
region B's AllGather completes.
"""

import numpy as np

FEAT = 128
N_CORES = 8
SPLIT_BLK = 31        # blocks 0..30 -> table region A, 31.. -> region B
CHUNK_BLOCKS = 2
LO_LAG = 4            # lo-gather lookahead, in chunks


# ---------------------------------------------------------------- host side

def preprocess(edge_index: np.ndarray, n_nodes: int, n_cores: int = N_CORES,
               chunk_blocks: int = CHUNK_BLOCKS):
    """Uniform SPMD tile schedule + per-core gather/metadata arrays."""
    src = np.asarray(edge_index[0], dtype=np.int64)
    dst = np.asarray(edge_index[1], dtype=np.int64)
    loops = np.arange(n_nodes, dtype=np.int64)
    s = np.concatenate([src, loops])
    d = np.concatenate([dst, loops])
    deg = np.bincount(d, minlength=n_nodes).astype(np.float64)  # >= 1
    dinv = (1.0 / np.sqrt(deg)).astype(np.float32)

    S_pc = n_nodes // n_cores
    assert S_pc * n_cores == n_nodes
    nblocks = (S_pc + 127) // 128
    split_blk = min(SPLIT_BLK, (nblocks + 1) // 2 + 6)
    ROWS_A = min(split_blk * 128, S_pc)      # local rows in table region A
    ROWS_B = S_pc - ROWS_A
    assert n_cores * ROWS_A < 32768 and n_cores * ROWS_B < 32768

    # node -> table row (region, row): node = q*S_pc + p
    q = s // S_pc
    p = s - q * S_pc
    in_a = p < ROWS_A
    trow = np.where(in_a, q * ROWS_A + p, q * ROWS_B + (p - ROWS_A))

    core_of = d // S_pc
    dloc = d - core_of * S_pc
    blk = dloc // 128
    dst_local = (dloc - blk * 128).astype(np.int32)
    is_hi = (~in_a).astype(np.int64)

    counts = np.zeros((n_cores, nblocks, 2), dtype=np.int64)
    np.add.at(counts, (core_of, blk, is_hi), 1)
    T_lo = -(-counts[:, :, 0].max(axis=0) // 128)
    T_hi = -(-counts[:, :, 1].max(axis=0) // 128)

    order = np.lexsort((is_hi, blk, core_of))
    trow_o, dl_o = trow[order], dst_local[order]
    co_o, blk_o, hi_o = core_of[order], blk[order], is_hi[order]

    n_tiles_lo = int(T_lo.sum())
    n_tiles_hi = int(T_hi.sum())
    cum_lo = np.concatenate([[0], np.cumsum(T_lo)])
    cum_hi = np.concatenate([[0], np.cumsum(T_hi)])

    key = (co_o * nblocks + blk_o) * 2 + hi_o
    bounds = np.searchsorted(key, np.arange(n_cores * nblocks * 2 + 1))
    per_core = []
    for r in range(n_cores):
        idx_lo = np.zeros(128 * n_tiles_lo, dtype=np.int16)
        idx_hi = np.zeros(128 * n_tiles_hi, dtype=np.int16)
        dloc_lo = np.full((128, max(n_tiles_lo, 1)), -1.0, dtype=np.float32)
        dloc_hi = np.full((128, max(n_tiles_hi, 1)), -1.0, dtype=np.float32)
        for b in range(nblocks):
            for h in range(2):
                k = (r * nblocks + b) * 2 + h
                lo_, hi_ = bounds[k], bounds[k + 1]
                cnt = hi_ - lo_
                t0 = cum_lo[b] if h == 0 else cum_hi[b]
                iarr = idx_lo if h == 0 else idx_hi
                darr = dloc_lo if h == 0 else dloc_hi
                iarr[128 * t0: 128 * t0 + cnt] = trow_o[lo_:hi_].astype(np.int16)
                e = np.arange(cnt)
                darr[e % 128, t0 + e // 128] = dl_o[lo_:hi_]

        def wrap(a):  # [n] -> [128, n//16]; idx i at [i%16 + 16k, i//16]
            n = len(a)
            if n == 0:
                return np.zeros((128, 1), dtype=np.int16)
            w = a.reshape(n // 16, 16).T
            return np.tile(w, (8, 1)).copy()

        # per-block dinv of the core's local (destination) nodes
        dv = np.ones(nblocks * 128, dtype=np.float32)
        dv[:S_pc] = dinv[r * S_pc:(r + 1) * S_pc]
        per_core.append(dict(
            idx_lo=wrap(idx_lo), idx_hi=wrap(idx_hi),
            dloc_lo=dloc_lo, dloc_hi=dloc_hi,
            dinv_blk=dv.reshape(nblocks, 128).T.copy(),  # [128, nblocks]
        ))

    chunks = [list(range(c, min(c + chunk_blocks, nblocks)))
              for c in range(0, nblocks, chunk_blocks)]
    sched = dict(
        n_nodes=n_nodes, n_cores=n_cores, S_pc=S_pc, nblocks=nblocks,
        split_blk=split_blk, ROWS_A=ROWS_A, ROWS_B=ROWS_B,
        T_lo=T_lo.astype(int), T_hi=T_hi.astype(int),
        cum_lo=cum_lo.astype(int), cum_hi=cum_hi.astype(int),
        n_tiles_lo=n_tiles_lo, n_tiles_hi=n_tiles_hi, chunks=chunks,
    )
    return sched, per_core


def make_inputs(sched, per_core, x, Ws, bs):
    n_cores, S_pc = sched["n_cores"], sched["S_pc"]
    iota = np.tile(np.arange(128, dtype=np.float32)[None, :], (128, 1))
    in_maps = []
    for r in range(n_cores):
        m = dict(
            x_shard=np.ascontiguousarray(x[r * S_pc:(r + 1) * S_pc]).astype(np.float32),
            iota=iota,
            idx_lo=per_core[r]["idx_lo"], idx_hi=per_core[r]["idx_hi"],
            dloc_lo=per_core[r]["dloc_lo"], dloc_hi=per_core[r]["dloc_hi"],
            dinv_blk=per_core[r]["dinv_blk"],
        )
        for i, (W, b) in enumerate(zip(Ws, bs)):
            m[f"W{i}"] = np.asarray(W).astype(np.float16)
            m[f"b{i}"] = np.tile(np.asarray(b, dtype=np.float32)[None, :], (128, 1))
        in_maps.append(m)
    return in_maps


# ---------------------------------------------------------------- device side

def build_nc(sched, fos=(128, 128, 64), n_cores=None, model=False,
             compile=True, single_packet=False, max_gather_idx=None,
             scratch=None, reps=1, probe=None,
             swdge_queues=4, agg_bufs=4, lo_bufs=LO_LAG + 2, hi_bufs=4,
             lo_lag=LO_LAG):
    """model=True: single-core cost-model variant (AllGather replaced by a
    local DMA) for TimelineSim.  reps>1 replicates the whole pipeline for
    on-hardware delta timing; the per-pipeline S-tile builds are re-issued
    every rep so the marginal rep is a faithful single-pipeline time."""
    import concourse.bacc as bacc
    import concourse.tile as tile
    import concourse.mybir as mybir

    f16, f32, i16 = mybir.dt.float16, mybir.dt.float32, mybir.dt.int16
    f8 = mybir.dt.float8e4
    N, S_pc = sched["n_nodes"], sched["S_pc"]
    nblocks = sched["nblocks"]
    ROWS_A = sched["ROWS_A"]
    split_blk = sched["split_blk"]
    T_lo, T_hi = sched["T_lo"], sched["T_hi"]
    cum_lo, cum_hi = sched["cum_lo"], sched["cum_hi"]
    NT_lo, NT_hi = sched["n_tiles_lo"], sched["n_tiles_hi"]
    chunks = sched["chunks"]
    n_cores = n_cores or sched["n_cores"]
    n_layers = len(fos)
    NA = n_cores * ROWS_A            # rows in table region A
    NB = N - NA
    assert NB > 0, "need a non-empty table region B"

    kw = {}
    if scratch:
        kw["dynamic_dma_scratch_size"] = scratch
    if swdge_queues > 1:
        kw["num_swdge_queues"] = swdge_queues
    nc = bacc.Bacc("TRN2", target_bir_lowering=False, debug=False,
                   num_devices=n_cores, **kw)

    x_shard = nc.dram_tensor("x_shard", [S_pc, FEAT], f32, kind="ExternalInput")
    iota_in = nc.dram_tensor("iota", [128, 128], f32, kind="ExternalInput")
    idx_lo_in = nc.dram_tensor("idx_lo", [128, max(NT_lo * 8, 1)], i16, kind="ExternalInput")
    idx_hi_in = nc.dram_tensor("idx_hi", [128, max(NT_hi * 8, 1)], i16, kind="ExternalInput")
    dloc_lo_in = nc.dram_tensor("dloc_lo", [128, max(NT_lo, 1)], f32, kind="ExternalInput")
    dloc_hi_in = nc.dram_tensor("dloc_hi", [128, max(NT_hi, 1)], f32, kind="ExternalInput")
    dinv_in = nc.dram_tensor("dinv_blk", [128, nblocks], f32, kind="ExternalInput")
    W_in = [nc.dram_tensor(f"W{i}", [FEAT, fos[i]], f16, kind="ExternalInput")
            for i in range(n_layers)]
    b_in = [nc.dram_tensor(f"b{i}", [128, fos[i]], f32, kind="ExternalInput")
            for i in range(n_layers)]
    y_out = nc.dram_tensor("y", [S_pc, fos[-1]], f32, kind="ExternalOutput")

    rg = [list(range(n_cores))]

    with tile.TileContext(nc) as tc:
        with (
            tc.tile_pool(name="const", bufs=1) as cpool,
            tc.tile_pool(name="sb", bufs=3) as sb,
            tc.tile_pool(name="mlo", bufs=lo_bufs) as mlo,
            tc.tile_pool(name="mhi", bufs=hi_bufs) as mhi,
            tc.tile_pool(name="psum_agg", bufs=agg_bufs, space="PSUM") as psum_agg,
            tc.tile_pool(name="psum_mm", bufs=2, space="PSUM") as psum_mm,
            tc.tile_pool(name="dram", bufs=2, space="DRAM") as dram,
        ):
            # --- constants, loaded once
            iota = cpool.tile([128, 128], f32)
            nc.sync.dma_start(out=iota[:], in_=iota_in[:])
            idx_lo = cpool.tile([128, max(NT_lo * 8, 1)], i16)
            nc.sync.dma_start(out=idx_lo[:], in_=idx_lo_in[:])
            idx_hi = cpool.tile([128, max(NT_hi * 8, 1)], i16)
            nc.sync.dma_start(out=idx_hi[:], in_=idx_hi_in[:])
            dloc_lo = cpool.tile([128, max(NT_lo, 1)], f32)
            nc.sync.dma_start(out=dloc_lo[:], in_=dloc_lo_in[:])
            dloc_hi = cpool.tile([128, max(NT_hi, 1)], f32)
            nc.sync.dma_start(out=dloc_hi[:], in_=dloc_hi_in[:])
            dinv = cpool.tile([128, nblocks], f32)
            nc.sync.dma_start(out=dinv[:], in_=dinv_in[:])
            Wt, bt = [], []
            for i in range(n_layers):
                w = cpool.tile([FEAT, fos[i]], f16, tag=f"W{i}")
                nc.sync.dma_start(out=w[:], in_=W_in[i][:])
                Wt.append(w)
                b = cpool.tile([128, fos[i]], f32, tag=f"b{i}")
                nc.sync.dma_start(out=b[:], in_=b_in[i][:])
                bt.append(b)

            # persistent SBUF cache of all one-hot S tiles (fp8): lo tiles at
            # col t*128, hi tiles at (NT_lo+t)*128
            s_cache = cpool.tile([128, (NT_lo + NT_hi) * 128], f8, tag="scache")

            def S_of(half, col):
                o = (col if half == 0 else NT_lo + col) * 128
                return s_cache[:, o:o + 128]

            def build_S(half, t0, nt):
                """One DVE op: S[:, e, d] = (iota[d] == dloc[e, t0+t]) for nt
                tiles of one half, written straight into the fp8 cache."""
                dl = dloc_lo if half == 0 else dloc_hi
                o = (t0 if half == 0 else NT_lo + t0) * 128
                out = s_cache[:, o:o + nt * 128].rearrange(
                    "p (t f) -> p t f", f=128)
                in0 = iota[:].rearrange("p (t f) -> p t f", t=1).broadcast_to(
                    [128, nt, 128])
                in1 = dl[:, t0:t0 + nt].rearrange(
                    "p (t f) -> p t f", f=1).broadcast_to([128, nt, 128])
                nc.vector.tensor_tensor(out, in0, in1, mybir.AluOpType.is_equal)

            def emit_gather(M, src_ap, idx_sb, t0, nt, queues):
                if probe == "nogather":
                    nc.sync.dma_start(out=M[:],
                                      in_=src_ap[0:128 * nt, :].rearrange(
                                          "(p t) f -> p t f", p=128))
                    return
                step = nt if not max_gather_idx else max(1, max_gather_idx // 128)
                for s0 in range(0, nt, step):
                    sn = min(step, nt - s0)
                    o16 = (t0 + s0) * 8
                    queues[0] = (queues[0] + 1) % len(queues[1])
                    nc.gpsimd.dma_gather(
                        out_ap=M[:, s0:s0 + sn, :], in_ap=src_ap,
                        idxs_ap=idx_sb[:, o16:o16 + sn * 8],
                        num_idxs=128 * sn, num_idxs_reg=128 * sn,
                        elem_size=FEAT, single_packet=single_packet,
                        queue_num=queues[1][queues[0]])

            qlo = [0, [0, 1]]
            qhi = [0, [2, 3]]
            if swdge_queues < 4:
                qlo = qhi = [0, list(range(swdge_queues))]

            def gather_lo(chunk):
                ctlo = int(sum(T_lo[b] for b in chunk))
                if not ctlo:
                    return None
                M = mlo.tile([128, ctlo, FEAT], f16, tag="Mlo")
                emit_gather(M, tbl_a[:], idx_lo, int(cum_lo[chunk[0]]), ctlo,
                            qlo)
                return M

            for rep in range(reps):
                # --- layer 0 input: scale by dinv and cast to fp16
                ag_in_a = dram.tile([ROWS_A, FEAT], f16, tag="ag_in_a")
                ag_in_b = dram.tile([S_pc - ROWS_A, FEAT], f16, tag="ag_in_b")
                for b in range(nblocks):
                    r0, r1 = b * 128, min((b + 1) * 128, S_pc)
                    rows = r1 - r0
                    xt = sb.tile([128, FEAT], f32, tag="xcast_in")
                    nc.sync.dma_start(out=xt[:rows, :], in_=x_shard[r0:r1, :])
                    xh = sb.tile([128, FEAT], f16, tag="xcast_out")
                    nc.vector.tensor_scalar(
                        xh[:rows, :], xt[:rows, :], dinv[:rows, b:b + 1], None,
                        mybir.AluOpType.mult)
                    if b < split_blk:
                        nc.sync.dma_start(out=ag_in_a[r0:r1, :], in_=xh[:rows, :])
                    else:
                        nc.sync.dma_start(
                            out=ag_in_b[r0 - ROWS_A:r1 - ROWS_A, :],
                            in_=xh[:rows, :])

                for l in range(n_layers):
                    fo = fos[l]
                    # --- allgather: region A (blocks < split_blk), region B.
                    # A fires early; the lo gathers of the first LO_LAG chunks
                    # are emitted before B so the in-order gpsimd queue keeps
                    # working while B's collective waits on the layer tail.
                    tbl_a = dram.tile([NA, FEAT], f16, tag="tbl_a")
                    tbl_b = dram.tile([NB, FEAT], f16, tag="tbl_b")
                    if model:
                        nc.sync.dma_start(out=tbl_a[0:ROWS_A, :], in_=ag_in_a[:])
                    else:
                        nc.gpsimd.collective_compute(
                            "AllGather", mybir.AluOpType.bypass,
                            replica_groups=rg,
                            ins=[ag_in_a[:].opt()], outs=[tbl_a[:].opt()])
                    M_lo_q = [gather_lo(chunks[ci])
                              for ci in range(min(lo_lag, len(chunks)))]
                    if model:
                        nc.sync.dma_start(out=tbl_b[0:S_pc - ROWS_A, :], in_=ag_in_b[:])
                    else:
                        nc.gpsimd.collective_compute(
                            "AllGather", mybir.AluOpType.bypass,
                            replica_groups=rg,
                            ins=[ag_in_b[:].opt()], outs=[tbl_b[:].opt()])
                    if l + 1 < n_layers:
                        ag_in_a = dram.tile([ROWS_A, FEAT], f16, tag="ag_in_a")
                        ag_in_b = dram.tile([S_pc - ROWS_A, FEAT], f16, tag="ag_in_b")

                    for ci, chunk in enumerate(chunks):
                        if l == 0 and probe != "noS":
                            if T_lo[chunk[0]:chunk[-1] + 1].sum():
                                build_S(0, int(cum_lo[chunk[0]]),
                                        int(sum(T_lo[b] for b in chunk)))
                            if T_hi[chunk[0]:chunk[-1] + 1].sum():
                                build_S(1, int(cum_hi[chunk[0]]),
                                        int(sum(T_hi[b] for b in chunk)))
                        elif l == 0 and rep == 0 and ci == 0:
                            build_S(0, 0, 1)
                        cthi = int(sum(T_hi[b] for b in chunk))
                        M_hi = None
                        if cthi:
                            M_hi = mhi.tile([128, cthi, FEAT], f16, tag="Mhi")
                            emit_gather(M_hi, tbl_b[:], idx_hi,
                                        int(cum_hi[chunk[0]]), cthi, qhi)
                        if ci + lo_lag < len(chunks):
                            M_lo_q.append(gather_lo(chunks[ci + lo_lag]))
                        M_lo = M_lo_q.pop(0)
                        if probe == "only_gather":
                            continue
                        for b in chunk:
                            tiles = (
                                [(0, M_lo, cum_lo[b] - cum_lo[chunk[0]] + k,
                                  cum_lo[b] + k)
                                 for k in range(T_lo[b])] +
                                [(1, M_hi, cum_hi[b] - cum_hi[chunk[0]] + k,
                                  cum_hi[b] + k)
                                 for k in range(T_hi[b])])
                            agg = psum_agg.tile([128, 128], f32)
                            for j, (half, Mt, sl, col) in enumerate(tiles):
                                S = (S_of(half, int(col)) if probe != "noS"
                                     else s_cache[:, 0:128])
                                nc.tensor.matmul(agg[:], Mt[:, sl, :], S,
                                                 start=(j == 0),
                                                 stop=(j == len(tiles) - 1))
                            aggT = sb.tile([128, 128], f16, tag="aggT")
                            nc.scalar.copy(aggT[:], agg[:])
                            mm = psum_mm.tile([128, fo], f32, tag="mm")
                            nc.tensor.matmul(mm[:], aggT[:], Wt[l][:],
                                             start=True, stop=True)
                            r0, r1 = b * 128, min((b + 1) * 128, S_pc)
                            rows = r1 - r0
                            # hs = dinv[d]*mm + b
                            hs = sb.tile([128, fo], f32, tag="hsum")
                            nc.vector.tensor_scalar(
                                hs[:], mm[:], dinv[:, b:b + 1], None,
                                mybir.AluOpType.mult)
                            hb = sb.tile([128, fo], f32 if l + 1 == n_layers else f16,
                                         tag="hbias")
                            nc.vector.tensor_tensor(
                                hb[:], hs[:], bt[l][:], mybir.AluOpType.add)
                            if l + 1 < n_layers:
                                # t_next = relu(dinv[d] * hb)
                                h = sb.tile([128, fo], f16, tag="hout")
                                nc.scalar.activation(
                                    h[:], hb[:],
                                    mybir.ActivationFunctionType.Relu,
                                    scale=dinv[:, b:b + 1])
                                if b < split_blk:
                                    nc.sync.dma_start(out=ag_in_a[r0:r1, :],
                                                      in_=h[:rows, :])
                                else:
                                    nc.sync.dma_start(
                                        out=ag_in_b[r0 - ROWS_A:r1 - ROWS_A, :],
                                        in_=h[:rows, :])
                            else:
                                nc.sync.dma_start(out=y_out[r0:r1, :],
                                                  in_=hb[:rows, :])
    if compile:
        nc.compile()
    return nc


# ---------------------------------------------------------------- entry point

_CACHE = {}


def kernel(x, edge_index, W1, b1, W2, b2, W3, b3):
    import sys
    if "/opt/trn_rl_repo" not in sys.path:
        sys.path.insert(0, "/opt/trn_rl_repo")
    from concourse import bass_utils

    x = np.asarray(x)
    edge_index = np.asarray(edge_index)
    Ws = [np.asarray(W1), np.asarray(W2), np.asarray(W3)]
    bs = [np.asarray(b1), np.asarray(b2), np.asarray(b3)]
    n = x.shape[0]

    key = (n, edge_index.shape[1])
    if key in _CACHE and np.array_equal(_CACHE[key][0], edge_index):
        _, sched, per_core, nc = _CACHE[key]
    else:
        sched, per_core = preprocess(edge_index, n, N_CORES)
        nc = build_nc(sched, fos=(W1.shape[1], W2.shape[1], W3.shape[1]))
        _CACHE[key] = (edge_index.copy(), sched, per_core, nc)

    in_maps = make_inputs(sched, per_core, x, Ws, bs)
    res = bass_utils.run_bass_kernel_spmd(nc, in_maps,
                                          core_ids=list(range(N_CORES)))
    out = np.concatenate([res.results[r]["y"] for r in range(N_CORES)], axis=0)
    return out.astype(np.float32)


# revision 13
# speedup vs baseline: 1.4276x; 1.2090x over previous
"""3-layer GCN (PyG GCNConv semantics) on 8 Trainium2 NeuronCores.

Contract: kernel(**inputs) takes the FULL inputs (x [50000,128] f32,
edge_index [2,800000] int, W1/b1/W2/b2/W3/b3) and returns the FULL
output [50000, 64] f32.

Sharding: nodes are partitioned across the 8 cores by destination
(6250 rows each).  Per layer, each core scales its local rows by
dinv = deg^-1/2 and casts to fp16; an AllGather replicates the full
scaled node-feature table to every core's DRAM.  Each core then
computes its destination shard:

    agg[d,:]  = sum_{edges (s,d)} t[s,:]          (t = dinv * h, one-hot S)
    mm[d,:]   = (dinv[d] * agg[d,:]) @ W + b
    t_next[d] = relu(dinv[d] * mm[d,:])           (inner layers)
    y[d]      = mm[d,:]                           (last layer)

The per-edge norm dinv[s]*dinv[d] is recovered exactly by the source-side
pre-scale and destination-side post-scale, so the per-edge selector
matrices S[e,d] = (d == dst_local_e) are PURE one-hot {0,1}.  They depend
only on the (fixed) graph, so S tiles are built once per pipeline into a
persistent fp8 SBUF cache (~19 tiles per DVE op via stride-0 broadcast
is_equal) interleaved with the first layer's chunks, and reused by the
later layers; the PE aggregation reads them as the streaming rhs operand
(fp16 x fp8 matmul).

The sparse aggregation per 128-dst block: dma_gather of the 128 source
rows of each edge tile (fp16, 256B rows) and a PE matmul
agg[f,d] += M[e,f].T @ S[e,d] accumulated in PSUM over the ~18 edge
tiles of the block.  dma_gather indices are int16, so the table is
addressed as two regions (A: blocks < 31, B: rest); region A's AllGather
fires as soon as the A blocks finish, and the A-side ("lo") gathers of
the next layer run LO_LAG chunks ahead on their own SWDGE queues while
region B's AllGather completes.
"""

import numpy as np

FEAT = 128
N_CORES = 8
SPLIT_BLK = 31        # blocks 0..30 -> table region A, 31.. -> region B
CHUNK_BLOCKS = 2
LO_LAG = 4            # lo-gather lookahead, in chunks


# ---------------------------------------------------------------- host side

def preprocess(edge_index: np.ndarray, n_nodes: int, n_cores: int = N_CORES,
               chunk_blocks: int = CHUNK_BLOCKS):
    """Uniform SPMD tile schedule + per-core gather/metadata arrays."""
    src = np.asarray(edge_index[0], dtype=np.int64)
    dst = np.asarray(edge_index[1], dtype=np.int64)
    loops = np.arange(n_nodes, dtype=np.int64)
    s = np.concatenate([src, loops])
    d = np.concatenate([dst, loops])
    deg = np.bincount(d, minlength=n_nodes).astype(np.float64)  # >= 1
    dinv = (1.0 / np.sqrt(deg)).astype(np.float32)

    S_pc = n_nodes // n_cores
    assert S_pc * n_cores == n_nodes
    nblocks = (S_pc + 127) // 128
    split_blk = min(SPLIT_BLK, (nblocks + 1) // 2 + 6)
    ROWS_A = min(split_blk * 128, S_pc)      # local rows in table region A
    ROWS_B = S_pc - ROWS_A
    assert n_cores * ROWS_A < 32768 and n_cores * ROWS_B < 32768

    # node -> table row (region, row): node = q*S_pc + p
    q = s // S_pc
    p = s - q * S_pc
    in_a = p < ROWS_A
    trow = np.where(in_a, q * ROWS_A + p, q * ROWS_B + (p - ROWS_A))

    core_of = d // S_pc
    dloc = d - core_of * S_pc
    blk = dloc // 128
    dst_local = (dloc - blk * 128).astype(np.int32)
    is_hi = (~in_a).astype(np.int64)

    counts = np.zeros((n_cores, nblocks, 2), dtype=np.int64)
    np.add.at(counts, (core_of, blk, is_hi), 1)
    T_lo = -(-counts[:, :, 0].max(axis=0) // 128)
    T_hi = -(-counts[:, :, 1].max(axis=0) // 128)

    order = np.lexsort((is_hi, blk, core_of))
    trow_o, dl_o = trow[order], dst_local[order]
    co_o, blk_o, hi_o = core_of[order], blk[order], is_hi[order]

    n_tiles_lo = int(T_lo.sum())
    n_tiles_hi = int(T_hi.sum())
    cum_lo = np.concatenate([[0], np.cumsum(T_lo)])
    cum_hi = np.concatenate([[0], np.cumsum(T_hi)])

    key = (co_o * nblocks + blk_o) * 2 + hi_o
    bounds = np.searchsorted(key, np.arange(n_cores * nblocks * 2 + 1))
    per_core = []
    for r in range(n_cores):
        idx_lo = np.zeros(128 * n_tiles_lo, dtype=np.int16)
        idx_hi = np.zeros(128 * n_tiles_hi, dtype=np.int16)
        dloc_lo = np.full((128, max(n_tiles_lo, 1)), -1.0, dtype=np.float32)
        dloc_hi = np.full((128, max(n_tiles_hi, 1)), -1.0, dtype=np.float32)
        for b in range(nblocks):
            for h in range(2):
                k = (r * nblocks + b) * 2 + h
                lo_, hi_ = bounds[k], bounds[k + 1]
                cnt = hi_ - lo_
                t0 = cum_lo[b] if h == 0 else cum_hi[b]
                iarr = idx_lo if h == 0 else idx_hi
                darr = dloc_lo if h == 0 else dloc_hi
                iarr[128 * t0: 128 * t0 + cnt] = trow_o[lo_:hi_].astype(np.int16)
                e = np.arange(cnt)
                darr[e % 128, t0 + e // 128] = dl_o[lo_:hi_]

        def wrap(a):  # [n] -> [128, n//16]; idx i at [i%16 + 16k, i//16]
            n = len(a)
            if n == 0:
                return np.zeros((128, 1), dtype=np.int16)
            w = a.reshape(n // 16, 16).T
            return np.tile(w, (8, 1)).copy()

        # per-block dinv of the core's local (destination) nodes
        dv = np.ones(nblocks * 128, dtype=np.float32)
        dv[:S_pc] = dinv[r * S_pc:(r + 1) * S_pc]
        per_core.append(dict(
            idx_lo=wrap(idx_lo), idx_hi=wrap(idx_hi),
            dloc_lo=dloc_lo, dloc_hi=dloc_hi,
            dinv_blk=dv.reshape(nblocks, 128).T.copy(),  # [128, nblocks]
        ))

    chunks = [list(range(c, min(c + chunk_blocks, nblocks)))
              for c in range(0, nblocks, chunk_blocks)]
    sched = dict(
        n_nodes=n_nodes, n_cores=n_cores, S_pc=S_pc, nblocks=nblocks,
        split_blk=split_blk, ROWS_A=ROWS_A, ROWS_B=ROWS_B,
        T_lo=T_lo.astype(int), T_hi=T_hi.astype(int),
        cum_lo=cum_lo.astype(int), cum_hi=cum_hi.astype(int),
        n_tiles_lo=n_tiles_lo, n_tiles_hi=n_tiles_hi, chunks=chunks,
    )
    return sched, per_core


def make_inputs(sched, per_core, x, Ws, bs):
    n_cores, S_pc = sched["n_cores"], sched["S_pc"]
    iota = np.tile(np.arange(128, dtype=np.float32)[None, :], (128, 1))
    in_maps = []
    for r in range(n_cores):
        m = dict(
            x_shard=np.ascontiguousarray(x[r * S_pc:(r + 1) * S_pc]).astype(np.float32),
            iota=iota,
            idx_lo=per_core[r]["idx_lo"], idx_hi=per_core[r]["idx_hi"],
            dloc_lo=per_core[r]["dloc_lo"], dloc_hi=per_core[r]["dloc_hi"],
            dinv_blk=per_core[r]["dinv_blk"],
        )
        for i, (W, b) in enumerate(zip(Ws, bs)):
            m[f"W{i}"] = np.asarray(W).astype(np.float16)
            m[f"b{i}"] = np.tile(np.asarray(b, dtype=np.float32)[None, :], (128, 1))
        in_maps.append(m)
    return in_maps


# ---------------------------------------------------------------- device side

def build_nc(sched, fos=(128, 128, 64), n_cores=None, model=False,
             compile=True, single_packet=False, max_gather_idx=None,
             scratch=None, reps=1, probe=None,
             swdge_queues=4, agg_bufs=4, lo_bufs=LO_LAG + 2, hi_bufs=4,
             lo_lag=LO_LAG):
    """model=True: single-core cost-model variant (AllGather replaced by a
    local DMA) for TimelineSim.  reps>1 replicates the whole pipeline for
    on-hardware delta timing; the per-pipeline S-tile builds are re-issued
    every rep so the marginal rep is a faithful single-pipeline time."""
    import concourse.bacc as bacc
    import concourse.tile as tile
    import concourse.mybir as mybir

    f16, f32, i16 = mybir.dt.float16, mybir.dt.float32, mybir.dt.int16
    f8 = mybir.dt.float8e4
    N, S_pc = sched["n_nodes"], sched["S_pc"]
    nblocks = sched["nblocks"]
    ROWS_A = sched["ROWS_A"]
    split_blk = sched["split_blk"]
    T_lo, T_hi = sched["T_lo"], sched["T_hi"]
    cum_lo, cum_hi = sched["cum_lo"], sched["cum_hi"]
    NT_lo, NT_hi = sched["n_tiles_lo"], sched["n_tiles_hi"]
    chunks = sched["chunks"]
    n_cores = n_cores or sched["n_cores"]
    n_layers = len(fos)
    NA = n_cores * ROWS_A            # rows in table region A
    NB = N - NA
    assert NB > 0, "need a non-empty table region B"

    kw = {}
    if scratch:
        kw["dynamic_dma_scratch_size"] = scratch
    if swdge_queues > 1:
        kw["num_swdge_queues"] = swdge_queues
    nc = bacc.Bacc("TRN2", target_bir_lowering=False, debug=False,
                   num_devices=n_cores, **kw)

    x_shard = nc.dram_tensor("x_shard", [S_pc, FEAT], f32, kind="ExternalInput")
    iota_in = nc.dram_tensor("iota", [128, 128], f32, kind="ExternalInput")
    idx_lo_in = nc.dram_tensor("idx_lo", [128, max(NT_lo * 8, 1)], i16, kind="ExternalInput")
    idx_hi_in = nc.dram_tensor("idx_hi", [128, max(NT_hi * 8, 1)], i16, kind="ExternalInput")
    dloc_lo_in = nc.dram_tensor("dloc_lo", [128, max(NT_lo, 1)], f32, kind="ExternalInput")
    dloc_hi_in = nc.dram_tensor("dloc_hi", [128, max(NT_hi, 1)], f32, kind="ExternalInput")
    dinv_in = nc.dram_tensor("dinv_blk", [128, nblocks], f32, kind="ExternalInput")
    W_in = [nc.dram_tensor(f"W{i}", [FEAT, fos[i]], f16, kind="ExternalInput")
            for i in range(n_layers)]
    b_in = [nc.dram_tensor(f"b{i}", [128, fos[i]], f32, kind="ExternalInput")
            for i in range(n_layers)]
    y_out = nc.dram_tensor("y", [S_pc, fos[-1]], f32, kind="ExternalOutput")

    rg = [list(range(n_cores))]

    with tile.TileContext(nc) as tc:
        with (
            tc.tile_pool(name="const", bufs=1) as cpool,
            tc.tile_pool(name="sb", bufs=3) as sb,
            tc.tile_pool(name="mlo", bufs=lo_bufs) as mlo,
            tc.tile_pool(name="mhi", bufs=hi_bufs) as mhi,
            tc.tile_pool(name="psum_agg", bufs=agg_bufs, space="PSUM") as psum_agg,
            tc.tile_pool(name="psum_mm", bufs=2, space="PSUM") as psum_mm,
            tc.tile_pool(name="dram", bufs=2, space="DRAM") as dram,
        ):
            # --- constants, loaded once
            iota = cpool.tile([128, 128], f32)
            nc.sync.dma_start(out=iota[:], in_=iota_in[:])
            idx_lo = cpool.tile([128, max(NT_lo * 8, 1)], i16)
            nc.sync.dma_start(out=idx_lo[:], in_=idx_lo_in[:])
            idx_hi = cpool.tile([128, max(NT_hi * 8, 1)], i16)
            nc.sync.dma_start(out=idx_hi[:], in_=idx_hi_in[:])
            dloc_lo = cpool.tile([128, max(NT_lo, 1)], f32)
            nc.sync.dma_start(out=dloc_lo[:], in_=dloc_lo_in[:])
            dloc_hi = cpool.tile([128, max(NT_hi, 1)], f32)
            nc.sync.dma_start(out=dloc_hi[:], in_=dloc_hi_in[:])
            dinv = cpool.tile([128, nblocks], f32)
            nc.sync.dma_start(out=dinv[:], in_=dinv_in[:])
            Wt, bt = [], []
            for i in range(n_layers):
                w = cpool.tile([FEAT, fos[i]], f16, tag=f"W{i}")
                nc.sync.dma_start(out=w[:], in_=W_in[i][:])
                Wt.append(w)
                b = cpool.tile([128, fos[i]], f32, tag=f"b{i}")
                nc.sync.dma_start(out=b[:], in_=b_in[i][:])
                bt.append(b)

            # persistent SBUF cache of all one-hot S tiles (fp8): lo tiles at
            # col t*128, hi tiles at (NT_lo+t)*128
            s_cache = cpool.tile([128, (NT_lo + NT_hi) * 128], f8, tag="scache")

            def S_of(half, col):
                o = (col if half == 0 else NT_lo + col) * 128
                return s_cache[:, o:o + 128]

            def build_S(half, t0, nt):
                """One DVE op: S[:, e, d] = (iota[d] == dloc[e, t0+t]) for nt
                tiles of one half, written straight into the fp8 cache."""
                dl = dloc_lo if half == 0 else dloc_hi
                o = (t0 if half == 0 else NT_lo + t0) * 128
                out = s_cache[:, o:o + nt * 128].rearrange(
                    "p (t f) -> p t f", f=128)
                in0 = iota[:].rearrange("p (t f) -> p t f", t=1).broadcast_to(
                    [128, nt, 128])
                in1 = dl[:, t0:t0 + nt].rearrange(
                    "p (t f) -> p t f", f=1).broadcast_to([128, nt, 128])
                nc.vector.tensor_tensor(out, in0, in1, mybir.AluOpType.is_equal)

            def emit_gather(M, src_ap, idx_sb, t0, nt, queues):
                if probe == "nogather":
                    nc.sync.dma_start(out=M[:],
                                      in_=src_ap[0:128 * nt, :].rearrange(
                                          "(p t) f -> p t f", p=128))
                    return
                step = nt if not max_gather_idx else max(1, max_gather_idx // 128)
                for s0 in range(0, nt, step):
                    sn = min(step, nt - s0)
                    o16 = (t0 + s0) * 8
                    queues[0] = (queues[0] + 1) % len(queues[1])
                    nc.gpsimd.dma_gather(
                        out_ap=M[:, s0:s0 + sn, :], in_ap=src_ap,
                        idxs_ap=idx_sb[:, o16:o16 + sn * 8],
                        num_idxs=128 * sn, num_idxs_reg=128 * sn,
                        elem_size=FEAT, single_packet=single_packet,
                        queue_num=queues[1][queues[0]])

            qlo = qhi = [0, list(range(swdge_queues))]

            def gather_lo(chunk):
                ctlo = int(sum(T_lo[b] for b in chunk))
                if not ctlo:
                    return None
                M = mlo.tile([128, ctlo, FEAT], f16, tag="Mlo")
                emit_gather(M, tbl_a[:], idx_lo, int(cum_lo[chunk[0]]), ctlo,
                            qlo)
                return M

            for rep in range(reps):
                # --- layer 0 input: scale by dinv and cast to fp16
                ag_in_a = dram.tile([ROWS_A, FEAT], f16, tag="ag_in_a")
                ag_in_b = dram.tile([S_pc - ROWS_A, FEAT], f16, tag="ag_in_b")
                for b in range(nblocks):
                    r0, r1 = b * 128, min((b + 1) * 128, S_pc)
                    rows = r1 - r0
                    xt = sb.tile([128, FEAT], f32, tag="xcast_in")
                    nc.sync.dma_start(out=xt[:rows, :], in_=x_shard[r0:r1, :])
                    xh = sb.tile([128, FEAT], f16, tag="xcast_out")
                    nc.vector.tensor_scalar(
                        xh[:rows, :], xt[:rows, :], dinv[:rows, b:b + 1], None,
                        mybir.AluOpType.mult)
                    if b < split_blk:
                        nc.sync.dma_start(out=ag_in_a[r0:r1, :], in_=xh[:rows, :])
                    else:
                        nc.sync.dma_start(
                            out=ag_in_b[r0 - ROWS_A:r1 - ROWS_A, :],
                            in_=xh[:rows, :])

                for l in range(n_layers):
                    fo = fos[l]
                    # --- allgather: region A (blocks < split_blk), region B.
                    # A fires early; the lo gathers of the first LO_LAG chunks
                    # are emitted before B so the in-order gpsimd queue keeps
                    # working while B's collective waits on the layer tail.
                    tbl_a = dram.tile([NA, FEAT], f16, tag="tbl_a")
                    tbl_b = dram.tile([NB, FEAT], f16, tag="tbl_b")
                    if model:
                        nc.sync.dma_start(out=tbl_a[0:ROWS_A, :], in_=ag_in_a[:])
                    else:
                        nc.gpsimd.collective_compute(
                            "AllGather", mybir.AluOpType.bypass,
                            replica_groups=rg,
                            ins=[ag_in_a[:].opt()], outs=[tbl_a[:].opt()])
                    M_lo_q = [gather_lo(chunks[ci])
                              for ci in range(min(lo_lag, len(chunks)))]
                    if model:
                        nc.sync.dma_start(out=tbl_b[0:S_pc - ROWS_A, :], in_=ag_in_b[:])
                    else:
                        nc.gpsimd.collective_compute(
                            "AllGather", mybir.AluOpType.bypass,
                            replica_groups=rg,
                            ins=[ag_in_b[:].opt()], outs=[tbl_b[:].opt()])
                    if l + 1 < n_layers:
                        ag_in_a = dram.tile([ROWS_A, FEAT], f16, tag="ag_in_a")
                        ag_in_b = dram.tile([S_pc - ROWS_A, FEAT], f16, tag="ag_in_b")

                    for ci, chunk in enumerate(chunks):
                        if l == 0 and probe != "noS":
                            if T_lo[chunk[0]:chunk[-1] + 1].sum():
                                build_S(0, int(cum_lo[chunk[0]]),
                                        int(sum(T_lo[b] for b in chunk)))
                            if T_hi[chunk[0]:chunk[-1] + 1].sum():
                                build_S(1, int(cum_hi[chunk[0]]),
                                        int(sum(T_hi[b] for b in chunk)))
                        elif l == 0 and rep == 0 and ci == 0:
                            build_S(0, 0, 1)
                        cthi = int(sum(T_hi[b] for b in chunk))
                        M_hi = None
                        if cthi:
                            M_hi = mhi.tile([128, cthi, FEAT], f16, tag="Mhi")
                            emit_gather(M_hi, tbl_b[:], idx_hi,
                                        int(cum_hi[chunk[0]]), cthi, qhi)
                        if ci + lo_lag < len(chunks):
                            M_lo_q.append(gather_lo(chunks[ci + lo_lag]))
                        M_lo = M_lo_q.pop(0)
                        if probe == "only_gather":
                            continue
                        for b in chunk:
                            tiles = (
                                [(0, M_lo, cum_lo[b] - cum_lo[chunk[0]] + k,
                                  cum_lo[b] + k)
                                 for k in range(T_lo[b])] +
                                [(1, M_hi, cum_hi[b] - cum_hi[chunk[0]] + k,
                                  cum_hi[b] + k)
                                 for k in range(T_hi[b])])
                            agg = psum_agg.tile([128, 128], f32)
                            for j, (half, Mt, sl, col) in enumerate(tiles):
                                S = (S_of(half, int(col)) if probe != "noS"
                                     else s_cache[:, 0:128])
                                nc.tensor.matmul(agg[:], Mt[:, sl, :], S,
                                                 start=(j == 0),
                                                 stop=(j == len(tiles) - 1))
                            aggT = sb.tile([128, 128], f16, tag="aggT")
                            nc.scalar.copy(aggT[:], agg[:])
                            mm = psum_mm.tile([128, fo], f32, tag="mm")
                            nc.tensor.matmul(mm[:], aggT[:], Wt[l][:],
                                             start=True, stop=True)
                            r0, r1 = b * 128, min((b + 1) * 128, S_pc)
                            rows = r1 - r0
                            # hs = dinv[d]*mm + b
                            hs = sb.tile([128, fo], f32, tag="hsum")
                            nc.vector.tensor_scalar(
                                hs[:], mm[:], dinv[:, b:b + 1], None,
                                mybir.AluOpType.mult)
                            hb = sb.tile([128, fo], f32 if l + 1 == n_layers else f16,
                                         tag="hbias")
                            nc.vector.tensor_tensor(
                                hb[:], hs[:], bt[l][:], mybir.AluOpType.add)
                            if l + 1 < n_layers:
                                # t_next = relu(dinv[d] * hb)
                                h = sb.tile([128, fo], f16, tag="hout")
                                nc.scalar.activation(
                                    h[:], hb[:],
                                    mybir.ActivationFunctionType.Relu,
                                    scale=dinv[:, b:b + 1])
                                if b < split_blk:
                                    nc.sync.dma_start(out=ag_in_a[r0:r1, :],
                                                      in_=h[:rows, :])
                                else:
                                    nc.sync.dma_start(
                                        out=ag_in_b[r0 - ROWS_A:r1 - ROWS_A, :],
                                        in_=h[:rows, :])
                            else:
                                nc.sync.dma_start(out=y_out[r0:r1, :],
                                                  in_=hb[:rows, :])
    if compile:
        nc.compile()
    return nc


# ---------------------------------------------------------------- entry point

_CACHE = {}


def kernel(x, edge_index, W1, b1, W2, b2, W3, b3):
    import sys
    if "/opt/trn_rl_repo" not in sys.path:
        sys.path.insert(0, "/opt/trn_rl_repo")
    from concourse import bass_utils

    x = np.asarray(x)
    edge_index = np.asarray(edge_index)
    Ws = [np.asarray(W1), np.asarray(W2), np.asarray(W3)]
    bs = [np.asarray(b1), np.asarray(b2), np.asarray(b3)]
    n = x.shape[0]

    key = (n, edge_index.shape[1])
    if key in _CACHE and np.array_equal(_CACHE[key][0], edge_index):
        _, sched, per_core, nc = _CACHE[key]
    else:
        sched, per_core = preprocess(edge_index, n, N_CORES)
        nc = build_nc(sched, fos=(W1.shape[1], W2.shape[1], W3.shape[1]))
        _CACHE[key] = (edge_index.copy(), sched, per_core, nc)

    in_maps = make_inputs(sched, per_core, x, Ws, bs)
    res = bass_utils.run_bass_kernel_spmd(nc, in_maps,
                                          core_ids=list(range(N_CORES)))
    out = np.concatenate([res.results[r]["y"] for r in range(N_CORES)], axis=0)
    return out.astype(np.float32)


# revision 14
# speedup vs baseline: 1.5732x; 1.1020x over previous
"""3-layer GCN (PyG GCNConv semantics) on 8 Trainium2 NeuronCores.

Contract: kernel(**inputs) takes the FULL inputs (x [50000,128] f32,
edge_index [2,800000] int, W1/b1/W2/b2/W3/b3) and returns the FULL
output [50000, 64] f32.

Sharding: nodes are partitioned across the 8 cores by destination
(6250 rows each).  Per layer, each core scales its local rows by
dinv = deg^-1/2 and casts to fp16; an AllGather replicates the full
scaled node-feature table to every core's DRAM.  Each core then
computes its destination shard:

    agg[d,:]  = sum_{edges (s,d)} t[s,:]          (t = dinv * h, one-hot S)
    mm[d,:]   = (dinv[d] * agg[d,:]) @ W + b
    t_next[d] = relu(dinv[d] * mm[d,:])           (inner layers)
    y[d]      = mm[d,:]                           (last layer)

The per-edge norm dinv[s]*dinv[d] is recovered exactly by the source-side
pre-scale and destination-side post-scale, so the per-edge selector
matrices S[e,d] = (d == dst_local_e) are PURE one-hot {0,1}.  They depend
only on the (fixed) graph, so S tiles are built once per pipeline into a
persistent fp8 SBUF cache (~19 tiles per DVE op via stride-0 broadcast
is_equal) interleaved with the first layer's chunks, and reused by the
later layers; the PE aggregation reads them as the streaming rhs operand
(fp16 x fp8 matmul).

The sparse aggregation per 128-dst block: dma_gather of the 128 source
rows of each edge tile (fp16, 256B rows) and a PE matmul
agg[f,d] += M[e,f].T @ S[e,d] accumulated in PSUM over the ~18 edge
tiles of the block.  dma_gather indices are int16, so the table is
addressed as two regions (A: blocks < 31, B: rest); region A's AllGather
fires as soon as the A blocks finish, and the A-side ("lo") gathers of
the next layer run LO_LAG chunks ahead on their own SWDGE queues while
region B's AllGather completes.
"""

import numpy as np

FEAT = 128
N_CORES = 8
SPLIT_BLK = 25        # blocks 0..30 -> table region A, 31.. -> region B
CHUNK_BLOCKS = 2
LO_LAG = 4            # lo-gather lookahead, in chunks


# ---------------------------------------------------------------- host side

def preprocess(edge_index: np.ndarray, n_nodes: int, n_cores: int = N_CORES,
               chunk_blocks: int = CHUNK_BLOCKS):
    """Uniform SPMD tile schedule + per-core gather/metadata arrays."""
    src = np.asarray(edge_index[0], dtype=np.int64)
    dst = np.asarray(edge_index[1], dtype=np.int64)
    loops = np.arange(n_nodes, dtype=np.int64)
    s = np.concatenate([src, loops])
    d = np.concatenate([dst, loops])
    deg = np.bincount(d, minlength=n_nodes).astype(np.float64)  # >= 1
    dinv = (1.0 / np.sqrt(deg)).astype(np.float32)

    S_pc = n_nodes // n_cores
    assert S_pc * n_cores == n_nodes
    nblocks = (S_pc + 127) // 128
    split_blk = min(SPLIT_BLK, (nblocks + 1) // 2 + 6)
    ROWS_A = min(split_blk * 128, S_pc)      # local rows in table region A
    ROWS_B = S_pc - ROWS_A
    assert n_cores * ROWS_A < 32768 and n_cores * ROWS_B < 32768

    # node -> table row (region, row): node = q*S_pc + p
    q = s // S_pc
    p = s - q * S_pc
    in_a = p < ROWS_A
    trow = np.where(in_a, q * ROWS_A + p, q * ROWS_B + (p - ROWS_A))

    core_of = d // S_pc
    dloc = d - core_of * S_pc
    blk = dloc // 128
    dst_local = (dloc - blk * 128).astype(np.int32)
    is_hi = (~in_a).astype(np.int64)

    counts = np.zeros((n_cores, nblocks, 2), dtype=np.int64)
    np.add.at(counts, (core_of, blk, is_hi), 1)
    T_lo = -(-counts[:, :, 0].max(axis=0) // 128)
    T_hi = -(-counts[:, :, 1].max(axis=0) // 128)

    order = np.lexsort((is_hi, blk, core_of))
    trow_o, dl_o = trow[order], dst_local[order]
    co_o, blk_o, hi_o = core_of[order], blk[order], is_hi[order]

    n_tiles_lo = int(T_lo.sum())
    n_tiles_hi = int(T_hi.sum())
    cum_lo = np.concatenate([[0], np.cumsum(T_lo)])
    cum_hi = np.concatenate([[0], np.cumsum(T_hi)])

    key = (co_o * nblocks + blk_o) * 2 + hi_o
    bounds = np.searchsorted(key, np.arange(n_cores * nblocks * 2 + 1))
    per_core = []
    for r in range(n_cores):
        idx_lo = np.zeros(128 * n_tiles_lo, dtype=np.int16)
        idx_hi = np.zeros(128 * n_tiles_hi, dtype=np.int16)
        dloc_lo = np.full((128, max(n_tiles_lo, 1)), -1.0, dtype=np.float32)
        dloc_hi = np.full((128, max(n_tiles_hi, 1)), -1.0, dtype=np.float32)
        for b in range(nblocks):
            for h in range(2):
                k = (r * nblocks + b) * 2 + h
                lo_, hi_ = bounds[k], bounds[k + 1]
                cnt = hi_ - lo_
                t0 = cum_lo[b] if h == 0 else cum_hi[b]
                iarr = idx_lo if h == 0 else idx_hi
                darr = dloc_lo if h == 0 else dloc_hi
                iarr[128 * t0: 128 * t0 + cnt] = trow_o[lo_:hi_].astype(np.int16)
                e = np.arange(cnt)
                darr[e % 128, t0 + e // 128] = dl_o[lo_:hi_]

        def wrap(a):  # [n] -> [128, n//16]; idx i at [i%16 + 16k, i//16]
            n = len(a)
            if n == 0:
                return np.zeros((128, 1), dtype=np.int16)
            w = a.reshape(n // 16, 16).T
            return np.tile(w, (8, 1)).copy()

        # per-block dinv of the core's local (destination) nodes
        dv = np.ones(nblocks * 128, dtype=np.float32)
        dv[:S_pc] = dinv[r * S_pc:(r + 1) * S_pc]
        per_core.append(dict(
            idx_lo=wrap(idx_lo), idx_hi=wrap(idx_hi),
            dloc_lo=dloc_lo, dloc_hi=dloc_hi,
            dinv_blk=dv.reshape(nblocks, 128).T.copy(),  # [128, nblocks]
        ))

    chunks = [list(range(c, min(c + chunk_blocks, nblocks)))
              for c in range(0, nblocks, chunk_blocks)]
    sched = dict(
        n_nodes=n_nodes, n_cores=n_cores, S_pc=S_pc, nblocks=nblocks,
        split_blk=split_blk, ROWS_A=ROWS_A, ROWS_B=ROWS_B,
        T_lo=T_lo.astype(int), T_hi=T_hi.astype(int),
        cum_lo=cum_lo.astype(int), cum_hi=cum_hi.astype(int),
        n_tiles_lo=n_tiles_lo, n_tiles_hi=n_tiles_hi, chunks=chunks,
    )
    return sched, per_core


def make_inputs(sched, per_core, x, Ws, bs):
    n_cores, S_pc = sched["n_cores"], sched["S_pc"]
    iota = np.tile(np.arange(128, dtype=np.float32)[None, :], (128, 1))
    in_maps = []
    for r in range(n_cores):
        m = dict(
            x_shard=np.ascontiguousarray(x[r * S_pc:(r + 1) * S_pc]).astype(np.float32),
            iota=iota,
            idx_lo=per_core[r]["idx_lo"], idx_hi=per_core[r]["idx_hi"],
            dloc_lo=per_core[r]["dloc_lo"], dloc_hi=per_core[r]["dloc_hi"],
            dinv_blk=per_core[r]["dinv_blk"],
        )
        for i, (W, b) in enumerate(zip(Ws, bs)):
            m[f"W{i}"] = np.asarray(W).astype(np.float16)
            m[f"b{i}"] = np.tile(np.asarray(b, dtype=np.float32)[None, :], (128, 1))
        in_maps.append(m)
    return in_maps


# ---------------------------------------------------------------- device side

def build_nc(sched, fos=(128, 128, 64), n_cores=None, model=False,
             compile=True, single_packet=False, max_gather_idx=None,
             scratch=None, reps=1, probe=None,
             swdge_queues=4, agg_bufs=4, lo_bufs=LO_LAG + 2, hi_bufs=4,
             lo_lag=LO_LAG):
    """model=True: single-core cost-model variant (AllGather replaced by a
    local DMA) for TimelineSim.  reps>1 replicates the whole pipeline for
    on-hardware delta timing; the per-pipeline S-tile builds are re-issued
    every rep so the marginal rep is a faithful single-pipeline time."""
    import concourse.bacc as bacc
    import concourse.tile as tile
    import concourse.mybir as mybir

    f16, f32, i16 = mybir.dt.float16, mybir.dt.float32, mybir.dt.int16
    f8 = mybir.dt.float8e4
    N, S_pc = sched["n_nodes"], sched["S_pc"]
    nblocks = sched["nblocks"]
    ROWS_A = sched["ROWS_A"]
    split_blk = sched["split_blk"]
    T_lo, T_hi = sched["T_lo"], sched["T_hi"]
    cum_lo, cum_hi = sched["cum_lo"], sched["cum_hi"]
    NT_lo, NT_hi = sched["n_tiles_lo"], sched["n_tiles_hi"]
    chunks = sched["chunks"]
    n_cores = n_cores or sched["n_cores"]
    n_layers = len(fos)
    NA = n_cores * ROWS_A            # rows in table region A
    NB = N - NA
    assert NB > 0, "need a non-empty table region B"

    kw = {}
    if scratch:
        kw["dynamic_dma_scratch_size"] = scratch
    if swdge_queues > 1:
        kw["num_swdge_queues"] = swdge_queues
    nc = bacc.Bacc("TRN2", target_bir_lowering=False, debug=False,
                   num_devices=n_cores, **kw)

    x_shard = nc.dram_tensor("x_shard", [S_pc, FEAT], f32, kind="ExternalInput")
    iota_in = nc.dram_tensor("iota", [128, 128], f32, kind="ExternalInput")
    idx_lo_in = nc.dram_tensor("idx_lo", [128, max(NT_lo * 8, 1)], i16, kind="ExternalInput")
    idx_hi_in = nc.dram_tensor("idx_hi", [128, max(NT_hi * 8, 1)], i16, kind="ExternalInput")
    dloc_lo_in = nc.dram_tensor("dloc_lo", [128, max(NT_lo, 1)], f32, kind="ExternalInput")
    dloc_hi_in = nc.dram_tensor("dloc_hi", [128, max(NT_hi, 1)], f32, kind="ExternalInput")
    dinv_in = nc.dram_tensor("dinv_blk", [128, nblocks], f32, kind="ExternalInput")
    W_in = [nc.dram_tensor(f"W{i}", [FEAT, fos[i]], f16, kind="ExternalInput")
            for i in range(n_layers)]
    b_in = [nc.dram_tensor(f"b{i}", [128, fos[i]], f32, kind="ExternalInput")
            for i in range(n_layers)]
    y_out = nc.dram_tensor("y", [S_pc, fos[-1]], f32, kind="ExternalOutput")

    rg = [list(range(n_cores))]

    with tile.TileContext(nc) as tc:
        with (
            tc.tile_pool(name="const", bufs=1) as cpool,
            tc.tile_pool(name="sb", bufs=3) as sb,
            tc.tile_pool(name="mlo", bufs=lo_bufs) as mlo,
            tc.tile_pool(name="mhi", bufs=hi_bufs) as mhi,
            tc.tile_pool(name="psum_agg", bufs=agg_bufs, space="PSUM") as psum_agg,
            tc.tile_pool(name="psum_mm", bufs=2, space="PSUM") as psum_mm,
            tc.tile_pool(name="dram", bufs=2, space="DRAM") as dram,
        ):
            # --- constants, loaded once
            iota = cpool.tile([128, 128], f32)
            nc.sync.dma_start(out=iota[:], in_=iota_in[:])
            idx_lo = cpool.tile([128, max(NT_lo * 8, 1)], i16)
            nc.sync.dma_start(out=idx_lo[:], in_=idx_lo_in[:])
            idx_hi = cpool.tile([128, max(NT_hi * 8, 1)], i16)
            nc.sync.dma_start(out=idx_hi[:], in_=idx_hi_in[:])
            dloc_lo = cpool.tile([128, max(NT_lo, 1)], f32)
            nc.sync.dma_start(out=dloc_lo[:], in_=dloc_lo_in[:])
            dloc_hi = cpool.tile([128, max(NT_hi, 1)], f32)
            nc.sync.dma_start(out=dloc_hi[:], in_=dloc_hi_in[:])
            dinv = cpool.tile([128, nblocks], f32)
            nc.sync.dma_start(out=dinv[:], in_=dinv_in[:])
            Wt, bt = [], []
            for i in range(n_layers):
                w = cpool.tile([FEAT, fos[i]], f16, tag=f"W{i}")
                nc.sync.dma_start(out=w[:], in_=W_in[i][:])
                Wt.append(w)
                b = cpool.tile([128, fos[i]], f32, tag=f"b{i}")
                nc.sync.dma_start(out=b[:], in_=b_in[i][:])
                bt.append(b)

            # persistent SBUF cache of all one-hot S tiles (fp8): lo tiles at
            # col t*128, hi tiles at (NT_lo+t)*128
            s_cache = cpool.tile([128, (NT_lo + NT_hi) * 128], f8, tag="scache")

            def S_of(half, col):
                o = (col if half == 0 else NT_lo + col) * 128
                return s_cache[:, o:o + 128]

            def build_S(half, t0, nt):
                """One DVE op: S[:, e, d] = (iota[d] == dloc[e, t0+t]) for nt
                tiles of one half, written straight into the fp8 cache."""
                dl = dloc_lo if half == 0 else dloc_hi
                o = (t0 if half == 0 else NT_lo + t0) * 128
                out = s_cache[:, o:o + nt * 128].rearrange(
                    "p (t f) -> p t f", f=128)
                in0 = iota[:].rearrange("p (t f) -> p t f", t=1).broadcast_to(
                    [128, nt, 128])
                in1 = dl[:, t0:t0 + nt].rearrange(
                    "p (t f) -> p t f", f=1).broadcast_to([128, nt, 128])
                nc.vector.tensor_tensor(out, in0, in1, mybir.AluOpType.is_equal)

            def emit_gather(M, src_ap, idx_sb, t0, nt, queues):
                if probe == "nogather":
                    nc.sync.dma_start(out=M[:],
                                      in_=src_ap[0:128 * nt, :].rearrange(
                                          "(p t) f -> p t f", p=128))
                    return
                step = nt if not max_gather_idx else max(1, max_gather_idx // 128)
                for s0 in range(0, nt, step):
                    sn = min(step, nt - s0)
                    o16 = (t0 + s0) * 8
                    queues[0] = (queues[0] + 1) % len(queues[1])
                    nc.gpsimd.dma_gather(
                        out_ap=M[:, s0:s0 + sn, :], in_ap=src_ap,
                        idxs_ap=idx_sb[:, o16:o16 + sn * 8],
                        num_idxs=128 * sn, num_idxs_reg=128 * sn,
                        elem_size=FEAT, single_packet=single_packet,
                        queue_num=queues[1][queues[0]])

            qlo = qhi = [0, list(range(swdge_queues))]

            def gather_lo(chunk):
                ctlo = int(sum(T_lo[b] for b in chunk))
                if not ctlo:
                    return None
                M = mlo.tile([128, ctlo, FEAT], f16, tag="Mlo")
                emit_gather(M, tbl_a[:], idx_lo, int(cum_lo[chunk[0]]), ctlo,
                            qlo)
                return M

            for rep in range(reps):
                # --- layer 0 input: scale by dinv and cast to fp16
                ag_in_a = dram.tile([ROWS_A, FEAT], f16, tag="ag_in_a")
                ag_in_b = dram.tile([S_pc - ROWS_A, FEAT], f16, tag="ag_in_b")
                for b in range(nblocks):
                    r0, r1 = b * 128, min((b + 1) * 128, S_pc)
                    rows = r1 - r0
                    xt = sb.tile([128, FEAT], f32, tag="xcast_in")
                    nc.sync.dma_start(out=xt[:rows, :], in_=x_shard[r0:r1, :])
                    xh = sb.tile([128, FEAT], f16, tag="xcast_out")
                    nc.vector.tensor_scalar(
                        xh[:rows, :], xt[:rows, :], dinv[:rows, b:b + 1], None,
                        mybir.AluOpType.mult)
                    if b < split_blk:
                        nc.sync.dma_start(out=ag_in_a[r0:r1, :], in_=xh[:rows, :])
                    else:
                        nc.sync.dma_start(
                            out=ag_in_b[r0 - ROWS_A:r1 - ROWS_A, :],
                            in_=xh[:rows, :])

                for l in range(n_layers):
                    fo = fos[l]
                    # --- allgather: region A (blocks < split_blk), region B.
                    # A fires early; the lo gathers of the first LO_LAG chunks
                    # are emitted before B so the in-order gpsimd queue keeps
                    # working while B's collective waits on the layer tail.
                    tbl_a = dram.tile([NA, FEAT], f16, tag="tbl_a")
                    tbl_b = dram.tile([NB, FEAT], f16, tag="tbl_b")
                    if model:
                        nc.sync.dma_start(out=tbl_a[0:ROWS_A, :], in_=ag_in_a[:])
                    else:
                        nc.gpsimd.collective_compute(
                            "AllGather", mybir.AluOpType.bypass,
                            replica_groups=rg,
                            ins=[ag_in_a[:].opt()], outs=[tbl_a[:].opt()])
                    M_lo_q = [gather_lo(chunks[ci])
                              for ci in range(min(lo_lag, len(chunks)))]
                    if model:
                        nc.sync.dma_start(out=tbl_b[0:S_pc - ROWS_A, :], in_=ag_in_b[:])
                    else:
                        nc.gpsimd.collective_compute(
                            "AllGather", mybir.AluOpType.bypass,
                            replica_groups=rg,
                            ins=[ag_in_b[:].opt()], outs=[tbl_b[:].opt()])
                    if l + 1 < n_layers:
                        ag_in_a = dram.tile([ROWS_A, FEAT], f16, tag="ag_in_a")
                        ag_in_b = dram.tile([S_pc - ROWS_A, FEAT], f16, tag="ag_in_b")

                    for ci, chunk in enumerate(chunks):
                        if l == 0 and probe != "noS":
                            if T_lo[chunk[0]:chunk[-1] + 1].sum():
                                build_S(0, int(cum_lo[chunk[0]]),
                                        int(sum(T_lo[b] for b in chunk)))
                            if T_hi[chunk[0]:chunk[-1] + 1].sum():
                                build_S(1, int(cum_hi[chunk[0]]),
                                        int(sum(T_hi[b] for b in chunk)))
                        elif l == 0 and rep == 0 and ci == 0:
                            build_S(0, 0, 1)
                        cthi = int(sum(T_hi[b] for b in chunk))
                        M_hi = None
                        if cthi:
                            M_hi = mhi.tile([128, cthi, FEAT], f16, tag="Mhi")
                            emit_gather(M_hi, tbl_b[:], idx_hi,
                                        int(cum_hi[chunk[0]]), cthi, qhi)
                        if ci + lo_lag < len(chunks):
                            M_lo_q.append(gather_lo(chunks[ci + lo_lag]))
                        M_lo = M_lo_q.pop(0)
                        if probe == "only_gather":
                            continue
                        for b in chunk:
                            tiles = (
                                [(0, M_lo, cum_lo[b] - cum_lo[chunk[0]] + k,
                                  cum_lo[b] + k)
                                 for k in range(T_lo[b])] +
                                [(1, M_hi, cum_hi[b] - cum_hi[chunk[0]] + k,
                                  cum_hi[b] + k)
                                 for k in range(T_hi[b])])
                            agg = psum_agg.tile([128, 128], f32)
                            for j, (half, Mt, sl, col) in enumerate(tiles):
                                S = (S_of(half, int(col)) if probe != "noS"
                                     else s_cache[:, 0:128])
                                nc.tensor.matmul(agg[:], Mt[:, sl, :], S,
                                                 start=(j == 0),
                                                 stop=(j == len(tiles) - 1))
                            aggT = sb.tile([128, 128], f16, tag="aggT")
                            nc.scalar.copy(aggT[:], agg[:])
                            mm = psum_mm.tile([128, fo], f32, tag="mm")
                            nc.tensor.matmul(mm[:], aggT[:], Wt[l][:],
                                             start=True, stop=True)
                            r0, r1 = b * 128, min((b + 1) * 128, S_pc)
                            rows = r1 - r0
                            # hs = dinv[d]*mm + b
                            hs = sb.tile([128, fo], f32, tag="hsum")
                            nc.vector.tensor_scalar(
                                hs[:], mm[:], dinv[:, b:b + 1], None,
                                mybir.AluOpType.mult)
                            hb = sb.tile([128, fo], f32 if l + 1 == n_layers else f16,
                                         tag="hbias")
                            nc.vector.tensor_tensor(
                                hb[:], hs[:], bt[l][:], mybir.AluOpType.add)
                            if l + 1 < n_layers:
                                # t_next = relu(dinv[d] * hb)
                                h = sb.tile([128, fo], f16, tag="hout")
                                nc.scalar.activation(
                                    h[:], hb[:],
                                    mybir.ActivationFunctionType.Relu,
                                    scale=dinv[:, b:b + 1])
                                if b < split_blk:
                                    nc.sync.dma_start(out=ag_in_a[r0:r1, :],
                                                      in_=h[:rows, :])
                                else:
                                    nc.sync.dma_start(
                                        out=ag_in_b[r0 - ROWS_A:r1 - ROWS_A, :],
                                        in_=h[:rows, :])
                            else:
                                nc.sync.dma_start(out=y_out[r0:r1, :],
                                                  in_=hb[:rows, :])
    if compile:
        nc.compile()
    return nc


# ---------------------------------------------------------------- entry point

_CACHE = {}


def kernel(x, edge_index, W1, b1, W2, b2, W3, b3):
    import sys
    if "/opt/trn_rl_repo" not in sys.path:
        sys.path.insert(0, "/opt/trn_rl_repo")
    from concourse import bass_utils

    x = np.asarray(x)
    edge_index = np.asarray(edge_index)
    Ws = [np.asarray(W1), np.asarray(W2), np.asarray(W3)]
    bs = [np.asarray(b1), np.asarray(b2), np.asarray(b3)]
    n = x.shape[0]

    key = (n, edge_index.shape[1])
    if key in _CACHE and np.array_equal(_CACHE[key][0], edge_index):
        _, sched, per_core, nc = _CACHE[key]
    else:
        sched, per_core = preprocess(edge_index, n, N_CORES)
        nc = build_nc(sched, fos=(W1.shape[1], W2.shape[1], W3.shape[1]))
        _CACHE[key] = (edge_index.copy(), sched, per_core, nc)

    in_maps = make_inputs(sched, per_core, x, Ws, bs)
    res = bass_utils.run_bass_kernel_spmd(nc, in_maps,
                                          core_ids=list(range(N_CORES)))
    out = np.concatenate([res.results[r]["y"] for r in range(N_CORES)], axis=0)
    return out.astype(np.float32)
